# revision 22
# baseline (speedup 1.0000x reference)
"""Distributed sparse embedding lookup (mean combiner) on 8 Trainium2 cores.

Strategy (data-parallel over output rows, table replicated on every core):
  - Each core owns 1/8 of the output rows (13312 = 104*128). row_indices is
    sorted, so each core's keys are a contiguous slice of the input.
  - Keys are bucketed into 31 vocab windows of 32768 rows (dma_gather index
    tensors are int16). Within a window, keys split into occ0 (first
    occurrence of each (window, row)) and tail (repeat occurrences).
  - Slot stream: [w0-occ0 | w1-occ0 | ... ] then [w0-tail | w1-tail | ...],
    each block padded to a 128-slot multiple (gather fills pad slots with a
    repeated real index; their invc is 0 so they contribute nothing).
  - Device pipeline: dma_gather (fp32 rows, <=1024 idx / instr) -> DVE
    multiply by per-slot 1/count with fp32->bf16 convert -> dma_scatter_add
    in SBUF parity mode into bf16 accumulator pairs.
  - Scatter cost on the Pool engine scales with the accumulator AP size,
    not num_idxs, so scatters are few and huge: ONE scatter per window for
    occ0 (pad tokens target the discard slot 13440), then ~4 "round"
    scatters over the global tail region (round k scatters the k-th repeat
    of each row; other tokens target the discard slot).
  - Final merge: pair adds on DVE, then two strided bf16 DMAs into the
    [13568, 64] output. Host concatenates cores and upcasts to fp32.

All index preprocessing is host-side numpy; all table-data movement and
floating-point arithmetic run on the device.
"""
import numpy as np

_B, _S, _D = 4096, 26, 64
_V = 1_000_000
_M = 8
_R = _B * _S            # 106496 output rows
_RC = _R // _M          # 13312 rows per core = 104 slots * 128
_WIN = 32768
_NWIN = (_V + _WIN - 1) // _WIN      # 31
_ORC = _RC + 256        # +256 pad rows; discard slots 104/105
_NSLOT = _ORC // 128                 # 106 slots (53 even, 53 odd)
_PADROW = _RC + 128     # slot 105 -> discarded; safe scatter pad target
_BG = 1024              # max num_idxs per dma_gather (HW ring validated)
_NPAIR = 4              # accumulator pairs (independent WAW chains)
_DBUFS = 8              # data tile-pool depth
_SDELAY = 3             # windows between a window's last gather and scatter

_prog_cache = {}


def _cdiv(a, b):
    return (a + b - 1) // b


def _pack16(v, budget, pad):
    out = np.full(budget, pad, dtype=np.int16)
    out[: len(v)] = v
    return np.tile(out.reshape(-1, 16).T, (8, 1))


def _prep(values, row_indices):
    """Split keys per core into per-window occ0 streams + global tail region.

    Returns (shape, in_maps):
      shape = (occ0_tiles[w], tail_tiles[w], gather_pieces, n_rounds,
               occ0_nidx[w], round_nidx[k])  -- uniform across cores
      in_maps = per-core {gidx, sidx, invc}
    """
    values = np.asarray(values).astype(np.int64)
    row_indices = np.asarray(row_indices).astype(np.int64)
    if np.any(np.diff(row_indices) < 0):
        order = np.argsort(row_indices, kind="stable")
        values, row_indices = values[order], row_indices[order]
    bounds = np.searchsorted(row_indices, np.arange(_M + 1) * _RC)

    cores = []
    for c in range(_M):
        lo, hi = bounds[c], bounds[c + 1]
        v = values[lo:hi]
        r = row_indices[lo:hi] - c * _RC
        counts = np.bincount(r, minlength=_RC).astype(np.float32)
        invc = (1.0 / np.maximum(counts, 1.0))
        w = v // _WIN
        order = np.lexsort((r, w))
        vs, rs, ws = v[order], r[order], w[order]
        n = len(vs)
        # occurrence index within (w, r)
        key = ws * _RC + rs
        grp_start = np.r_[True, key[1:] != key[:-1]]
        start_pos = np.maximum.accumulate(np.where(grp_start, np.arange(n), 0))
        occ = np.arange(n) - start_pos
        # global tail-round index: j-th tail key of row r (first-fit level)
        is_tail = occ >= 1
        tail_idx = np.flatnonzero(is_tail)
        tr = rs[tail_idx]
        t_order = np.argsort(tr, kind="stable")
        t_sorted = tr[t_order]
        tg = np.r_[True, t_sorted[1:] != t_sorted[:-1]]
        t_start = np.maximum.accumulate(
            np.where(tg, np.arange(len(t_sorted)), 0))
        rounds_sorted = np.arange(len(t_sorted)) - t_start
        rounds_g = np.full(n, -1, dtype=np.int64)
        rounds_g[tail_idx[t_order]] = rounds_sorted
        cores.append(dict(vs=vs, rs=rs, ws=ws, occ=occ, invc=invc,
                          rounds=rounds_sorted, rounds_g=rounds_g))

    # per-window budgets (tiles), uniform across cores
    occ0_tiles, tail_tiles = [], []
    occ0_nidx = []
    for wi in range(_NWIN):
        n0max = n1max = 0
        for c in cores:
            m = c["ws"] == wi
            n0 = int((c["occ"][m] == 0).sum())
            n1 = int((c["occ"][m] >= 1).sum())
            n0max, n1max = max(n0max, n0), max(n1max, n1)
        occ0_tiles.append(max(_cdiv(n0max, 128), 1))
        tail_tiles.append(max(_cdiv(n1max, 128), 1))
        occ0_nidx.append(_cdiv(n0max, 16) * 16)
    n_rounds = 0
    for c in cores:
        if len(c["rounds"]):
            n_rounds = max(n_rounds, int(c["rounds"].max()) + 1)

    # gather piece plan: per window-occ0 block and per window-tail block,
    # split into <=_BG pieces at tile granularity (uniform: budget-based)
    pieces = []  # (window, tile_offset_in_stream, ntiles)
    toff = 0
    for wi in range(_NWIN):
        nt = occ0_tiles[wi]
        done = 0
        while done < nt:
            take = min(nt - done, _BG // 128)
            pieces.append((wi, toff + done, take))
            done += take
        toff += nt
    tail_base = toff
    for wi in range(_NWIN):
        nt = tail_tiles[wi]
        done = 0
        while done < nt:
            take = min(nt - done, _BG // 128)
            pieces.append((wi, toff + done, take))
            done += take
        toff += nt
    ntot = toff  # total tiles in stream

    # tail region span (tiles) and round num_idxs (uniform: worst core)
    tail_tiles_total = ntot - tail_base
    round_nidx = []
    for k in range(n_rounds):
        # num_idxs for round k: through the last round-k token (worst core)
        worst = 0
        for c in cores:
            # compute slot position of each tail key (filled below per core);
            # conservative: full tail span
            if (c["rounds"] == k).any():
                worst = tail_tiles_total * 128
        round_nidx.append(_cdiv(worst, 16) * 16)

    shape = (tuple(occ0_tiles), tuple(tail_tiles), tuple(
        (p[0], p[1], p[2]) for p in pieces), n_rounds,
        tuple(occ0_nidx), tuple(round_nidx), ntot, tail_base)

    in_maps = []
    for c in cores:
        gidx = np.zeros((ntot * 128,), dtype=np.int16)
        ivc = np.zeros((ntot * 128,), dtype=np.float32)
        # occ0 sidx per window; tail round sidx
        occ0_sidx = []
        tail_rows = np.full((tail_tiles_total * 128,), -1, dtype=np.int64)
        tail_round = np.full((tail_tiles_total * 128,), -1, dtype=np.int64)
        toff0 = 0
        tofft = tail_base
        for wi in range(_NWIN):
            m = c["ws"] == wi
            vs, rs, occ = c["vs"][m], c["rs"][m], c["occ"][m]
            rgm = c["rounds_g"][m]
            m0 = occ == 0
            v0, r0 = vs[m0], rs[m0]
            v1, r1 = vs[~m0], rs[~m0]
            rnd1 = rgm[~m0]
            # occ0 block
            base = toff0 * 128
            nt0 = occ0_tiles[wi]
            n0 = len(v0)
            gidx[base:base + n0] = (v0 - wi * _WIN).astype(np.int16)
            if n0 < nt0 * 128:
                fill = (v0[-1] - wi * _WIN) if n0 else 0
                gidx[base + n0:base + nt0 * 128] = np.int16(fill)
            ivc[base:base + n0] = c["invc"][r0]
            # sidx for this window's occ0 scatter
            sid = np.full((occ0_nidx[wi],), _PADROW, dtype=np.int16)
            sid[:n0] = r0.astype(np.int16)
            occ0_sidx.append(_pack16(sid, occ0_nidx[wi], np.int16(_PADROW)))
            toff0 += nt0
            # tail block
            baset = (tofft - tail_base) * 128
            nt1 = tail_tiles[wi]
            n1 = len(v1)
            gbase = tofft * 128
            gidx[gbase:gbase + n1] = (v1 - wi * _WIN).astype(np.int16)
            if n1 < nt1 * 128:
                fill = (v1[-1] - wi * _WIN) if n1 else 0
                gidx[gbase + n1:gbase + nt1 * 128] = np.int16(fill)
            ivc[gbase:gbase + n1] = c["invc"][r1]
            tail_rows[baset:baset + n1] = r1
            tail_round[baset:baset + n1] = rnd1
            tofft += nt1
        # round sidx over the whole tail region
        round_sidx = []
        for k in range(n_rounds):
            nk = round_nidx[k]
            sid = np.full((nk,), _PADROW, dtype=np.int16)
            sel = tail_round == k
            sid[np.flatnonzero(sel)] = tail_rows[sel].astype(np.int16)
            round_sidx.append(_pack16(sid, nk, np.int16(_PADROW)))
        sidx = np.concatenate(occ0_sidx + round_sidx, axis=1)
        in_maps.append({
            "gidx": np.ascontiguousarray(
                _pack16(gidx, ntot * 128, np.int16(0))),
            "sidx": np.ascontiguousarray(sidx),
            "invc": np.ascontiguousarray(ivc.reshape(-1, 128).T),
        })
    return shape, in_maps


def _build(shape, n_reps=1):
    from concourse import bacc, mybir, tile

    (occ0_tiles, tail_tiles, pieces, n_rounds, occ0_nidx, round_nidx,
     ntot, tail_base) = shape
    nc = bacc.Bacc(None, target_bir_lowering=False, debug=False,
                   num_swdge_queues=1)
    table = nc.dram_tensor("table", [_V, _D], mybir.dt.float32,
                           kind="ExternalInput")
    gidx = nc.dram_tensor("gidx", [128, ntot * 8], mybir.dt.int16,
                          kind="ExternalInput")
    stot = (sum(occ0_nidx) + sum(round_nidx)) // 16
    sidx = nc.dram_tensor("sidx", [128, stot], mybir.dt.int16,
                          kind="ExternalInput")
    invc = nc.dram_tensor("invc", [128, ntot], mybir.dt.float32,
                          kind="ExternalInput")
    out = nc.dram_tensor("out", [_ORC, _D], mybir.dt.bfloat16,
                         kind="ExternalOutput")
    HG = _NSLOT // 2   # 53 slots per parity

    with tile.TileContext(nc) as tc:
        with (
            tc.tile_pool(name="acc", bufs=1) as apool,
            tc.tile_pool(name="data", bufs=_DBUFS) as dpool,
            tc.tile_pool(name="meta", bufs=1) as mpool,
            tc.tile_pool(name="tail", bufs=1) as tpool,
        ):
            accs = []
            for p in range(_NPAIR):
                aa = apool.tile([128, HG, _D], mybir.dt.bfloat16, tag=f"aa{p}")
                ab = apool.tile([128, HG, _D], mybir.dt.bfloat16, tag=f"ab{p}")
                nc.vector.memset(aa[:], 0.0)
                nc.vector.memset(ab[:], 0.0)
                accs.append((aa, ab))

            gix = mpool.tile([128, ntot * 8], mybir.dt.int16, tag="gix")
            six = mpool.tile([128, stot], mybir.dt.int16, tag="six")
            ivx = mpool.tile([128, ntot], mybir.dt.float32, tag="ivx")
            nc.sync.dma_start(out=gix[:], in_=gidx[:])
            nc.sync.dma_start(out=six[:], in_=sidx[:])
            nc.sync.dma_start(out=ivx[:], in_=invc[:])

            tail_tot = ntot - tail_base
            tailbuf = tpool.tile([128, tail_tot, _D], mybir.dt.bfloat16,
                                 tag="tailbuf")

            for _rep in range(n_reps):
                chain = 0
                # occ0 scatter offsets in six (columns of 16)
                s_off = [0]
                for wi in range(_NWIN):
                    s_off.append(s_off[-1] + occ0_nidx[wi] // 16)
                win_buf = {}          # wi -> (scbuf, filled_tiles)

                def _scatter_occ0(wi):
                    nonlocal chain
                    scbuf, nt0 = win_buf.pop(wi)
                    n_idx = occ0_nidx[wi]
                    aa, ab = accs[chain % _NPAIR]
                    chain += 1
                    nc.gpsimd.dma_scatter_add(
                        out_ap=aa[:], in_ap=scbuf[:, :nt0, :],
                        idxs_ap=six[:, s_off[wi]:s_off[wi] + n_idx // 16],
                        num_idxs=n_idx, num_idxs_reg=n_idx,
                        elem_size=_D, queue_num=0, sbuf_tokens_per_rank=128,
                        parity_reg=0, out_ap_other=ab[:],
                    )

                # process pieces: gather -> DVE scale+convert into either the
                # window's contiguous scbuf (occ0) or the persistent tailbuf.
                # occ0 scatters issue _SDELAY windows late so their DVE input
                # is ready by the time Pool's in-order SEQ reaches them.
                pend = []
                cur_w = -1
                for wi, t0, nt in pieces:
                    is_tail = t0 >= tail_base
                    base = wi * _WIN
                    wsize = min(_WIN, _V - base)
                    gat = dpool.tile([128, nt, _D], mybir.dt.float32,
                                     tag="gat")
                    nidx = nt * 128
                    nc.gpsimd.dma_gather(
                        out_ap=gat[:], in_ap=table[base:base + wsize, :],
                        idxs_ap=gix[:, t0 * 8:t0 * 8 + nidx // 16],
                        num_idxs=nidx, num_idxs_reg=nidx,
                        elem_size=_D, queue_num=0,
                    )
                    if is_tail:
                        dst = tailbuf[:, t0 - tail_base:t0 - tail_base + nt, :]
                    else:
                        if wi != cur_w:
                            if cur_w >= 0:
                                pend.append(cur_w)
                            cur_w = wi
                            scb = dpool.tile(
                                [128, occ0_tiles[wi], _D],
                                mybir.dt.bfloat16, tag="scbuf")
                            win_buf[wi] = (scb, occ0_tiles[wi])
                            while len(pend) >= _SDELAY:
                                _scatter_occ0(pend.pop(0))
                        scb = win_buf[wi][0]
                        done = t0 - sum(occ0_tiles[:wi])
                        dst = scb[:, done:done + nt, :]
                    nc.vector.tensor_tensor(
                        out=dst, in0=gat[:],
                        in1=ivx[:, t0:t0 + nt, None].to_broadcast(
                            [128, nt, _D]),
                        op=mybir.AluOpType.mult,
                    )
                if cur_w >= 0:
                    pend.append(cur_w)
                for wi in pend:
                    _scatter_occ0(wi)

                # tail rounds over the whole tail region
                r_off = s_off[_NWIN]
                for k in range(n_rounds):
                    nk = round_nidx[k]
                    aa, ab = accs[chain % _NPAIR]
                    chain += 1
                    nc.gpsimd.dma_scatter_add(
                        out_ap=aa[:], in_ap=tailbuf[:],
                        idxs_ap=six[:, r_off:r_off + nk // 16],
                        num_idxs=nk, num_idxs_reg=nk,
                        elem_size=_D, queue_num=0, sbuf_tokens_per_rank=128,
                        parity_reg=0, out_ap_other=ab[:],
                    )
                    r_off += nk // 16

            # merge pairs into accs[0] and write out
            for par in range(2):
                acc0 = accs[0][par][:]
                for p in range(1, _NPAIR):
                    nc.vector.tensor_add(out=acc0, in0=acc0,
                                         in1=accs[p][par][:])
                out_view = out[:].rearrange("(s p) d -> p s d", p=128)
                nc.sync.dma_start(out=out_view[:, par::2, :], in_=acc0)
    nc.compile()
    return nc


def _state(values, row_indices, emb_table, n_reps=1):
    shape, in_maps = _prep(values, row_indices)
    key = (shape, n_reps)
    if key not in _prog_cache:
        _prog_cache[key] = _build(shape, n_reps=n_reps)
    nc = _prog_cache[key]
    table = np.ascontiguousarray(np.asarray(emb_table, dtype=np.float32))
    for m in in_maps:
        m["table"] = table
    return nc, in_maps


def kernel(values, row_indices, emb_table):
    from concourse.bass_utils import run_bass_kernel_spmd

    nc, in_maps = _state(values, row_indices, emb_table)
    res = run_bass_kernel_spmd(nc, in_maps, core_ids=list(range(_M)))
    full = np.concatenate(
        [np.asarray(res.results[c]["out"])[:_RC].astype(np.float32)
         for c in range(_M)], axis=0)
    return np.ascontiguousarray(full.reshape(_B, _S, _D))
